# revision 2
# baseline (speedup 1.0000x reference)
"""Trainium2 Bass kernel for single-head causal attention.

Problem: out = softmax(causal((x@Wq.T) @ (x@Wk.T).T / sqrt(C))) @ (x@Wv.T)
  x: [B=8, T=2048, C=1024] f32, Wq/Wk/Wv: [H=1024, C=1024] f32.

Sharding: data-parallel over batch B — each of the 8 NeuronCores computes one
batch element end-to-end; no collectives.

Per-core design (matmul operands bf16, fp32 PSUM accumulation):
  - Host precomputes M = Wq.T @ Wk (so S = x M x^T needs one projection
    u = x@M instead of separate q,k), pre-transposes x -> xT [C,T] bf16.
  - u-projection, then per s-band si: v-projection, S^T band matmuls
    (<=4 psum chunk chains), exp on ScalarE with 1/sqrt(C) folded in
    (diagonal 128 cols exp'd first so the causal mask-mul and the PV chain
    tail unblock early), then the PV accumulation for out tile ti=si.
  - Row sums l are accumulated as [1,T] rows via wide ones-stationary
    matmuls (40 instead of 136 one-column matmuls) and shipped to the host,
    which performs the final out_u / l normalization (exact: same bf16 P
    values are summed as the device PV uses).
"""

import sys

if "/opt/trn_rl_repo" not in sys.path:
    sys.path.insert(0, "/opt/trn_rl_repo")

from contextlib import ExitStack

import numpy as np
import ml_dtypes

import concourse.bacc as bacc
import concourse.mybir as mybir
from concourse.tile import TileContext
from concourse.bass_utils import run_bass_kernel_spmd
from concourse.masks import make_upper_triangular

bf16 = ml_dtypes.bfloat16

B, T, C, H = 8, 2048, 1024, 1024
PB = 128
NT = T // PB  # 16
NCC = C // PB  # 8
CH = 512
NTC = T // CH  # 4
NHC = H // CH  # 2
SCALE = float(C) ** -0.5
N_CORES = 8

_f32 = mybir.dt.float32
_bf = mybir.dt.bfloat16


def _emit_body(ctx, nc, tc, xT, m, wvT, out, lrow):
    const = ctx.enter_context(tc.tile_pool(name="const", bufs=1))
    mask = const.tile([PB, PB], _bf, name="mask")
    make_upper_triangular(nc, mask, val=1.0, diag=True)
    ones = const.tile([PB, 1], _bf, name="ones")
    nc.vector.memset(ones, 1.0)

    persist = ctx.enter_context(tc.tile_pool(name="persist", bufs=1))
    uT_sb = [persist.tile([PB, T], _bf, name=f"uT{i}", tag=f"uT{i}") for i in range(NCC)]
    v_sb = [persist.tile([PB, H], _bf, name=f"v{i}", tag=f"v{i}") for i in range(NT)]
    PT_sb = [
        persist.tile([PB, T - i * PB], _bf, name=f"PT{i}", tag=f"PT{i}")
        for i in range(NT)
    ]

    psum = ctx.enter_context(tc.tile_pool(name="psum", bufs=6, space="PSUM"))
    psum_l = ctx.enter_context(tc.tile_pool(name="psuml", bufs=2, space="PSUM"))
    ostage = ctx.enter_context(tc.tile_pool(name="ostage", bufs=3))

    xpool = ctx.enter_context(tc.tile_pool(name="xpool", bufs=1))
    xT_sb = [xpool.tile([PB, T], _bf, name=f"xT{i}", tag=f"xT{i}") for i in range(NCC)]

    with (
        tc.tile_pool(name="mpool", bufs=1) as mpool,
        tc.tile_pool(name="wvpool", bufs=1) as wvpool,
    ):
        m_sb = [mpool.tile([PB, C], _bf, name=f"m{i}", tag=f"m{i}") for i in range(NCC)]
        wv_sb = [
            wvpool.tile([PB, H], _bf, name=f"wv{i}", tag=f"wv{i}") for i in range(NCC)
        ]
        for i in range(NCC):
            nc.sync.dma_start(out=xT_sb[i], in_=xT[i * PB : (i + 1) * PB, :])
            nc.sync.dma_start(out=m_sb[i], in_=m[i * PB : (i + 1) * PB, :])
        for i in range(NCC):
            nc.sync.dma_start(out=wv_sb[i], in_=wvT[i * PB : (i + 1) * PB, :])

        # ---- u projection interleaved with v projections ----
        for hi in range(NCC):
            psU = [psum.tile([PB, CH], _f32, name=f"psU{j}", tag="mm") for j in range(NTC)]
            for ci in range(NCC):
                stat = m_sb[ci][:, hi * PB : (hi + 1) * PB]
                for j in range(NTC):
                    nc.tensor.matmul(
                        psU[j], stat, xT_sb[ci][:, j * CH : (j + 1) * CH],
                        start=(ci == 0), stop=(ci == NCC - 1),
                    )
            for j in range(NTC):
                nc.vector.tensor_copy(uT_sb[hi][:, j * CH : (j + 1) * CH], psU[j])


        for si in range(NT):
            base = si * PB
            psV = [
                psum.tile([PB, CH], _f32, name=f"psV{h}", tag="mm")
                for h in range(NHC)
            ]
            for ci in range(NCC):
                stat = xT_sb[ci][:, base : base + PB]
                for h in range(NHC):
                    nc.tensor.matmul(
                        psV[h], stat, wv_sb[ci][:, h * CH : (h + 1) * CH],
                        start=(ci == 0), stop=(ci == NCC - 1),
                    )
            for h in range(NHC):
                nc.vector.tensor_copy(v_sb[si][:, h * CH : (h + 1) * CH], psV[h])

        # ---- per s-band: S^T chunk-pairs + exp, PV, l-rows ----
        for si in range(NT):
            base = si * PB

            chunks = []
            t0 = base
            while t0 < T:
                t1 = min((t0 // CH + 1) * CH, T)
                chunks.append((t0, t1))
                t0 = t1
            first = True
            if True:
                pair = chunks
                psS = [
                    psum.tile([PB, t1 - t0], _f32, name=f"psS{j}", tag="mm")
                    for j, (t0, t1) in enumerate(pair)
                ]
                for ci in range(NCC):
                    stat = xT_sb[ci][:, base : base + PB]
                    for j, (t0, t1) in enumerate(pair):
                        nc.tensor.matmul(
                            psS[j], stat, uT_sb[ci][:, t0:t1],
                            start=(ci == 0), stop=(ci == NCC - 1),
                        )
                for j, (t0, t1) in enumerate(pair):
                    if first:
                        nc.scalar.activation(
                            out=PT_sb[si][:, 0:PB], in_=psS[j][:, 0:PB],
                            func=mybir.ActivationFunctionType.Exp, scale=SCALE,
                        )
                        nc.vector.tensor_mul(
                            PT_sb[si][:, 0:PB], PT_sb[si][:, 0:PB], mask
                        )
                        if t1 - t0 > PB:
                            nc.scalar.activation(
                                out=PT_sb[si][:, PB : t1 - base],
                                in_=psS[j][:, PB:],
                                func=mybir.ActivationFunctionType.Exp, scale=SCALE,
                            )
                        first = False
                    else:
                        nc.scalar.activation(
                            out=PT_sb[si][:, t0 - base : t1 - base], in_=psS[j],
                            func=mybir.ActivationFunctionType.Exp, scale=SCALE,
                        )

            # PV for ti = si: unnormalized out, DMA'd straight from PSUM
            ti, tb = si, base
            psO = [psum.tile([PB, CH], _f32, name=f"psO{h}", tag="mm") for h in range(NHC)]
            for sj in range(ti + 1):
                pt_blk = PT_sb[sj][:, tb - sj * PB : tb - sj * PB + PB]
                for h in range(NHC):
                    nc.tensor.matmul(
                        psO[h], pt_blk, v_sb[sj][:, h * CH : (h + 1) * CH],
                        start=(sj == 0), stop=(sj == ti),
                    )
            osb = ostage.tile([PB, H], _f32, name="osb", tag="osb")
            for h in range(NHC):
                nc.vector.tensor_copy(osb[:, h * CH : (h + 1) * CH], psO[h])
            nc.sync.dma_start(out=out[tb : tb + PB, :], in_=osb)

            # l-row chain for t-chunk tc completes once band si == 4*tc+3
            if si % 4 == 3:
                tc_i = si // 4
                tlo = tc_i * CH
                psLr = psum_l.tile([1, CH], _f32, name=f"psLr{tc_i}", tag="l")
                n_sj = 4 * tc_i + 4
                for sj in range(n_sj):
                    a = max(0, tlo - sj * PB)  # band-local col of chunk start
                    b = tlo + CH - sj * PB
                    c0 = max(0, sj * PB - tlo)  # chunk-local start col
                    nc.tensor.matmul(
                        psLr[:, c0:CH],
                        ones,
                        PT_sb[sj][:, a:b],
                        start=(sj == 0),
                        stop=(sj == n_sj - 1),
                        skip_group_check=(c0 > 0),
                    )
                lsb = ostage.tile([1, CH], _f32, name="lsb", tag="lsb")
                nc.vector.tensor_copy(lsb, psLr)
                nc.sync.dma_start(out=lrow[tc_i : tc_i + 1, :], in_=lsb)


def build(reps: int = 1, loop: int | None = None):
    nc = bacc.Bacc("TRN2", target_bir_lowering=False, debug=False, num_devices=N_CORES)
    xT = nc.dram_tensor("xT", [C, T], _bf, kind="ExternalInput").ap()
    m = nc.dram_tensor("m", [C, C], _bf, kind="ExternalInput").ap()
    wvT = nc.dram_tensor("wvT", [C, H], _bf, kind="ExternalInput").ap()
    out = nc.dram_tensor("out", [T, H], _f32, kind="ExternalOutput").ap()
    lrow = nc.dram_tensor("lrow", [NTC, CH], _f32, kind="ExternalOutput").ap()

    with TileContext(nc) as tc:
        if loop is not None:
            with tc.For_i(0, loop, 1):
                with ExitStack() as ctx:
                    _emit_body(ctx, nc, tc, xT, m, wvT, out, lrow)
        else:
            for _ in range(reps):
                with ExitStack() as ctx:
                    _emit_body(ctx, nc, tc, xT, m, wvT, out, lrow)
    nc.compile()
    return nc


_nc_cache = {}


def _get_nc(key=(1, None)):
    if key not in _nc_cache:
        reps, loop = key
        _nc_cache[key] = build(reps=reps, loop=loop)
    return _nc_cache[key]


def prep_in_maps(x, Wq, Wk, Wv):
    x = np.asarray(x, dtype=np.float32)
    Wq = np.asarray(Wq, dtype=np.float32)
    Wk = np.asarray(Wk, dtype=np.float32)
    Wv = np.asarray(Wv, dtype=np.float32)
    xTn = np.ascontiguousarray(x.transpose(0, 2, 1)).astype(bf16)
    M = (Wq.T @ Wk).astype(bf16)
    wvT = np.ascontiguousarray(Wv.T).astype(bf16)
    return [{"xT": xTn[b], "m": M, "wvT": wvT} for b in range(N_CORES)]


def kernel(x, Wq, Wk, Wv):
    assert np.asarray(x).shape == (B, T, C)
    nc = _get_nc()
    in_maps = prep_in_maps(x, Wq, Wk, Wv)
    res = run_bass_kernel_spmd(nc, in_maps, list(range(N_CORES)))
    outs = []
    for b in range(N_CORES):
        ou = res.results[b]["out"]
        l = res.results[b]["lrow"].reshape(T)
        outs.append(ou / l[:, None])
    return np.stack(outs, axis=0).astype(np.float32)
